# revision 6
# baseline (speedup 1.0000x reference)
"""Trainium2 Bass kernel for nn_ByteBitwiseFFN.

Reference semantics (per token, D=128 features):
  a = argmax(x[4:20]) + 16*argmax(x[20:36])
  b = argmax(x[36:52]) + 16*argmax(x[52:68])
  res = AND/OR/XOR LUT[a,b] picked by flags x[1]>0.5 / x[2]>0.5 / x[3]>0.5
        (priority AND, OR, XOR; XOR value also used when no flag set)
  active = (x[0]>=0.5) & any-flag; w = active ? 2 : 0
  out = x; out[68 + (res&15)] += w; out[84 + (res>>4)] += w

Key identities:
* Bitwise ops factor over nibbles, so the 256x256 LUTs are never needed:
  res&15 = op(a_lo, b_lo), res>>4 = op(a_hi, b_hi), and for 4-bit operands
  op(u, v) = alpha*(u+v) + beta*(u AND v) with (alpha, beta) =
  (0,1) AND / (1,-1) OR / (1,-2) XOR.  The AND is one int16 bitwise_and.
* Compare-free first-occurrence argmax via the bf16 bit pattern:
  d = max - x >= 0, and for non-negative bf16 the raw bit pattern is
  order-preserving with bits(0) == 0 and bits(d>0) >= 128 (values below
  1e-38 cannot occur: data gaps are > 1e-6).  So
  min over the field of (bitcast_i16(d) + n) == the argmax position n,
  computed entirely in int16.

Sharding: pure data parallel over tokens; each of the 8 cores gets
131072/8 = 16384 tokens as its own ExternalInput (plus two tiny
replicated constant tensors, DMA'd like inputs).

Engine choice (from per-op HW microbenchmarks): everything computes on
the Vector engine.  Offloading passes to GpSimd/ACT lowers DVE busy
time but concurrent GpSimd streaming makes co-scheduled DVE ops hit a
nondeterministic ~17x slow path and adds cross-engine bubbles; the
single-engine schedule overlaps DMA nearly perfectly and measured
fastest.  Small algebra runs as tensor_tensor against constant tiles
(tensor_scalar with strided inputs has the same slow-path hazard);
constants are shipped from the host as extra DMA inputs.  Chunks are
tapered [T/2, 3T/2 | 3T/2, T/2] to shorten pipeline fill/drain, and
each store is split in two for finer DMA interleave.
"""

import sys

if "/opt/trn_rl_repo" not in sys.path:
    sys.path.insert(0, "/opt/trn_rl_repo")

import numpy as np

B, S, D = 16, 8192, 128
N_CORES = 8
TOK = B * S                      # 131072 tokens
TOK_PER_CORE = TOK // N_CORES    # 16384
P = 128                          # SBUF partitions

OUT_LO, OUT_HI = 68, 84

T_CHUNK = 32
GROUP = 2
GT_ = GROUP * T_CHUNK

def const_layout(t_per_chunk=T_CHUNK, group=GROUP):
    """int16 constant block layout (per partition): n-pattern then 1/2/3/16.

    Chunks are tapered [T/2, 3T/2 | 3T/2, T/2]; the n-pattern must cover
    the largest chunk (3T/2).
    """
    GT = group * t_per_chunk
    t_max = t_per_chunk * 3 // 2
    idx_len = t_max * 64
    offs = {
        "idx": 0,
        "one": idx_len,
        "two": idx_len + GT,
        "three": idx_len + 2 * GT,
        "sixteen": idx_len + 3 * GT,
    }
    return offs, idx_len + 4 * GT


def make_const_inputs(t_per_chunk=T_CHUNK, group=GROUP):
    GT = group * t_per_chunk
    t_max = t_per_chunk * 3 // 2
    offs, ci_len = const_layout(t_per_chunk, group)
    ci = np.zeros((P, ci_len), np.int16)
    ci[:, 0 : t_max * 64] = np.tile(np.arange(16, dtype=np.int16), t_max * 4)
    for name, val in (("one", 1), ("two", 2), ("three", 3), ("sixteen", 16)):
        o = offs[name]
        ci[:, o : o + GT] = val
    cf = np.full((P, t_max * 4), 0.5, np.float32)
    return ci, cf


def build_program(tok_per_core=TOK_PER_CORE, t_per_chunk=T_CHUNK, group=GROUP):
    """Build + compile the single-core SPMD Bass program.

    The core's [tok_per_core, 128] slab is processed in chunks of 128*T
    tokens (contiguous DRAM block <-> SBUF tile [128, T*128]).  Heavy
    streaming passes run per chunk; small per-token algebra runs once per
    group of `group` chunks.
    """
    import concourse.bass as bass  # noqa: F401
    from concourse import bacc, mybir, tile

    f32 = mybir.dt.float32
    bf16 = mybir.dt.bfloat16
    i16 = mybir.dt.int16
    Op = mybir.AluOpType
    X = mybir.AxisListType.X

    T = t_per_chunk
    assert T % 2 == 0 and group == 2
    assert tok_per_core % (P * T * group) == 0
    n_groups = tok_per_core // (P * T * group)
    GT = group * T
    t_max = T * 3 // 2
    # tapered chunk sizes: small first chunk (fast pipeline fill) and small
    # last chunk (short drain); every group still spans GT tokens/partition
    sched = [[T // 2, t_max]] + [[t_max, T // 2] for _ in range(n_groups - 1)]
    if n_groups > 2:
        for gsz in sched[1:-1]:
            gsz[0] = T
            gsz[1] = T
    offs, ci_len = const_layout(t_per_chunk, group)

    nc = bacc.Bacc(
        "TRN2",
        target_bir_lowering=False,
        debug=False,
        enable_asserts=True,
        num_devices=N_CORES,
    )
    x_dram = nc.dram_tensor("x", [tok_per_core, D], f32, kind="ExternalInput").ap()
    ci_dram = nc.dram_tensor("ci", [P, ci_len], i16, kind="ExternalInput").ap()
    cf_dram = nc.dram_tensor("cf", [P, t_max * 4], f32, kind="ExternalInput").ap()
    y_dram = nc.dram_tensor("y", [tok_per_core, D], f32, kind="ExternalOutput").ap()

    with tile.TileContext(nc) as tc:
        with (
            tc.tile_pool(name="consts", bufs=1) as cpool,
            tc.tile_pool(name="xtiles", bufs=5) as xpool,
            tc.tile_pool(name="big", bufs=3) as bp,
            tc.tile_pool(name="small", bufs=3) as sp,
        ):
            v = nc.vector
            g = nc.gpsimd

            # --- constants (DMA'd from host, on the scalar HWDGE ring so the
            # sync ring starts streaming x immediately) ---------------------
            cit = cpool.tile([P, ci_len], i16)
            nc.scalar.dma_start(cit[:], ci_dram)
            cft = cpool.tile([P, t_max * 4], f32)
            nc.scalar.dma_start(cft[:], cf_dram)

            idxi_full_max = cit[:, 0 : t_max * 64]

            def c1(off):  # [P, GT, 1] int16 const view
                return cit[:, off : off + GT].unsqueeze(2)

            ones, twos, threes, sixteens = (
                c1(offs["one"]), c1(offs["two"]),
                c1(offs["three"]), c1(offs["sixteen"]),
            )
            half4_max = cft.rearrange("p (t f) -> p t f", f=4)

            tok0 = 0
            for gi in range(n_groups):
                xts = []
                # group result tiles (interleaved [t, 4]), int16
                am_all = sp.tile([P, GT * 4], i16, name="am_all")
                am4 = am_all.rearrange("p (t g) -> p t g", g=4)
                fl_all = sp.tile([P, GT * 4], i16, name="fl_all")
                fl4 = fl_all.rearrange("p (t f) -> p t f", f=4)

                tws = []
                tw = 0
                for ci in range(group):
                    Tc = sched[gi][ci]
                    tws.append(tw)
                    chunk_tok = P * Tc
                    xt = xpool.tile([P, t_max * D], f32, name="xt")[:, 0 : Tc * D]
                    xts.append(xt)
                    src = x_dram[tok0 : tok0 + chunk_tok, :].rearrange(
                        "(p t) f -> p (t f)", p=P
                    )
                    nc.sync.dma_start(xt[:], src)

                    x3 = xt.rearrange("p (t f) -> p t f", f=D)
                    nib = x3[:, :, 4:68].rearrange("p t (g n) -> p t g n", n=16)

                    # field max (exact, f32)
                    rmax = bp.tile([P, t_max * 4], f32, name="rmax")[:, 0 : Tc * 4]
                    rmax3 = rmax.rearrange("p (t g) -> p t g", g=4)
                    v.tensor_reduce(rmax3, nib, axis=X, op=Op.max)

                    # d = max - x >= 0, as bf16 (fused (-1*x) + max on DVE)
                    dsub = bp.tile([P, t_max * 64], bf16, name="dsub")[:, 0 : Tc * 64]
                    dsub4 = dsub.rearrange("p (t g n) -> p t g n", g=4, n=16)
                    v.scalar_tensor_tensor(
                        dsub4,
                        nib,
                        -1.0,
                        rmax3.unsqueeze(3).broadcast_to([P, Tc, 4, 16]),
                        Op.mult,
                        Op.add,
                    )

                    # cand = bits(d) + n  (int16; == n exactly at max positions)
                    cand = bp.tile([P, t_max * 64], i16, name="cand")[:, 0 : Tc * 64]
                    v.tensor_tensor(
                        cand[:], dsub[:].bitcast(i16), idxi_full_max[:, 0 : Tc * 64],
                        Op.add,
                    )

                    # per-field argmax position via a 16->8->4->2->1 min tree:
                    # int16 tensor_tensor runs in 2x perf mode, unlike the
                    # always-1x tensor_reduce (halves the cycles of this pass)
                    c4 = cand.rearrange("p (t g n) -> p t g n", g=4, n=16)
                    m8 = bp.tile([P, t_max * 32], i16, name="m8")[
                        :, 0 : Tc * 32
                    ].rearrange("p (t g n) -> p t g n", g=4, n=8)
                    v.tensor_tensor(m8, c4[:, :, :, 0:8], c4[:, :, :, 8:16], Op.min)
                    m4 = bp.tile([P, t_max * 16], i16, name="m4")[
                        :, 0 : Tc * 16
                    ].rearrange("p (t g n) -> p t g n", g=4, n=4)
                    v.tensor_tensor(m4, m8[:, :, :, 0:4], m8[:, :, :, 4:8], Op.min)
                    m2 = bp.tile([P, t_max * 8], i16, name="m2")[
                        :, 0 : Tc * 8
                    ].rearrange("p (t g n) -> p t g n", g=4, n=2)
                    v.tensor_tensor(m2, m4[:, :, :, 0:2], m4[:, :, :, 2:4], Op.min)
                    v.tensor_tensor(
                        am4[:, tw : tw + Tc, :].unsqueeze(3),
                        m2[:, :, :, 0:1],
                        m2[:, :, :, 1:2],
                        Op.min,
                    )
                    # flags (>= 0.5) for cols 0..3 as int16 0/1
                    v.tensor_tensor(
                        fl4[:, tw : tw + Tc, :],
                        x3[:, :, 0:4],
                        half4_max[:, 0:Tc, :],
                        Op.is_ge,
                    )
                    tok0 += chunk_tok
                    tw += Tc

                # --- per-token algebra for the group, all int16.  Lo and hi
                # halves ride together as a [P, GT, 2] pair per op (am4 fields
                # are a_lo, a_hi, b_lo, b_hi) — halves the small-op count.
                mk = fl4[:, :, 0:1]
                ia = fl4[:, :, 1:2]
                io = fl4[:, :, 2:3]
                ix = fl4[:, :, 3:4]
                amL = am4[:, :, 0:2]
                amR = am4[:, :, 2:4]

                def t1(nm):
                    t_ = sp.tile([P, GT], i16, name=nm)
                    return t_.unsqueeze(2)   # [P, GT, 1]

                def t2(nm):
                    t_ = sp.tile([P, GT * 2], i16, name=nm)
                    return t_.rearrange("p (t h) -> p t h", h=2)  # [P, GT, 2]

                alpha = t1("alpha")          # 1 - is_and
                v.tensor_tensor(alpha, ones, ia, Op.subtract)
                s1 = t1("s1")                # 3 - is_or
                v.tensor_tensor(s1, threes, io, Op.subtract)
                s3 = t1("s3")                # is_or - 2
                v.tensor_tensor(s3, io, twos, Op.subtract)
                s2 = t1("s2")
                v.tensor_tensor(s2, ia, s1, Op.mult)
                beta = t1("beta")            # 1 / -1 / -2
                v.tensor_tensor(beta, s2, s3, Op.add)
                or1 = t1("or1")
                v.tensor_tensor(or1, ia, io, Op.bitwise_or)
                or2 = t1("or2")
                v.tensor_tensor(or2, or1, ix, Op.bitwise_or)
                acti = t1("acti")            # active = mark & any-flag
                v.tensor_tensor(acti, mk, or2, Op.bitwise_and)
                act16 = t1("act16")
                v.tensor_tensor(act16, acti, sixteens, Op.mult)
                goff = t1("goff")            # 16*(1-active)
                v.tensor_tensor(goff, sixteens, act16, Op.subtract)

                qi = t2("qi")                # a AND b, both halves
                v.tensor_tensor(qi, amL, amR, Op.bitwise_and)
                ss = t2("ss")                # a + b
                v.tensor_tensor(ss, amL, amR, Op.add)
                c1_ = t2("c1")
                v.tensor_tensor(c1_, ss, alpha.broadcast_to([P, GT, 2]), Op.mult)
                c2 = t2("c2")
                v.tensor_tensor(c2, qi, beta.broadcast_to([P, GT, 2]), Op.mult)
                res2 = t2("res2")            # op(a, b), 0..15
                v.tensor_tensor(res2, c1_, c2, Op.add)
                resg2 = t2("resg2")          # pushed out of 0..15 if inactive
                v.tensor_tensor(resg2, res2, goff.broadcast_to([P, GT, 2]), Op.add)

                # one-hot over the contiguous 32-feature output block 68:100:
                # lane (h, n) = 1 iff resg2[h] == n  (the 2x rides the accum)
                eq2 = sp.tile([P, GT * 32], bf16, name="eq2")
                eq4 = eq2.rearrange("p (t h n) -> p t h n", h=2, n=16)
                v.tensor_tensor(
                    eq4,
                    idxi_full_max[:, 0 : GT * 32].rearrange(
                        "p (t h n) -> p t h n", h=2, n=16
                    ),
                    resg2.unsqueeze(3).broadcast_to([P, GT, 2, 16]),
                    Op.is_equal,
                )
                eq3 = eq2.rearrange("p (t k) -> p t k", k=32)

                # --- accumulate into x and store, per chunk ----------------
                stok0 = tok0 - P * GT
                for ci in range(group):
                    Tc = sched[gi][ci]
                    tw = tws[ci]
                    x3 = xts[ci].rearrange("p (t f) -> p t f", f=D)
                    xs = x3[:, :, OUT_LO : OUT_LO + 32]
                    v.scalar_tensor_tensor(
                        xs,
                        eq3[:, tw : tw + Tc, :],
                        2.0,
                        xs,
                        Op.mult,
                        Op.add,
                    )
                    # two half-stores: finer DMA interleave, shorter tail
                    half = Tc // 2
                    dst3 = y_dram[stok0 : stok0 + P * Tc, :].rearrange(
                        "(p t) f -> p t f", p=P
                    )
                    src3 = x3
                    # stores ride the scalar HWDGE ring (qActDynamicHW) so they
                    # don't serialize behind loads on qSyncDynamicHW — each HW
                    # ring tops out ~200 GB/s, together they reach the HBM limit
                    for s in range(2):
                        nc.scalar.dma_start(
                            dst3[:, s * half : (s + 1) * half, :],
                            src3[:, s * half : (s + 1) * half, :],
                        )
                    stok0 += P * Tc

    nc.compile()
    return nc


_compiled = None


def _get_compiled():
    global _compiled
    if _compiled is None:
        _compiled = build_program()
    return _compiled


def run_on_hw(nc, shards, trace=False, t_per_chunk=T_CHUNK, group=GROUP, **kw):
    from concourse.bass_utils import run_bass_kernel_spmd

    ci, cf = make_const_inputs(t_per_chunk, group)
    return run_bass_kernel_spmd(
        nc,
        [{"x": s, "ci": ci, "cf": cf} for s in shards],
        list(range(N_CORES)),
        trace=trace,
        **kw,
    )


def kernel(x_bd, and_table=None, or_table=None, xor_table=None):
    x = np.ascontiguousarray(np.asarray(x_bd, dtype=np.float32)).reshape(TOK, D)
    shards = [
        np.ascontiguousarray(x[c * TOK_PER_CORE : (c + 1) * TOK_PER_CORE])
        for c in range(N_CORES)
    ]
    nc = _get_compiled()
    res = run_on_hw(nc, shards)
    out = np.concatenate([res.results[c]["y"] for c in range(N_CORES)], axis=0)
    return out.reshape(B, S, D).astype(np.float32)



# revision 10
# speedup vs baseline: 1.2807x; 1.2807x over previous
"""Trainium2 Bass kernel for nn_ByteBitwiseFFN.

Reference semantics (per token, D=128 features):
  a = argmax(x[4:20]) + 16*argmax(x[20:36])
  b = argmax(x[36:52]) + 16*argmax(x[52:68])
  res = AND/OR/XOR LUT[a,b] picked by flags x[1]>0.5 / x[2]>0.5 / x[3]>0.5
        (priority AND, OR, XOR; XOR value also used when no flag set)
  active = (x[0]>=0.5) & any-flag; w = active ? 2 : 0
  out = x; out[68 + (res&15)] += w; out[84 + (res>>4)] += w

Key identities:
* Bitwise ops factor over nibbles, so the 256x256 LUTs are never needed:
  res&15 = op(a_lo, b_lo), res>>4 = op(a_hi, b_hi), and for 4-bit operands
  op(u, v) = alpha*(u+v) + beta*(u AND v) with (alpha, beta) =
  (0,1) AND / (1,-1) OR / (1,-2) XOR.  The AND is one int16 bitwise_and.
* Compare-free first-occurrence argmax via the bf16 bit pattern:
  d = max - x >= 0, and for non-negative bf16 the raw bit pattern is
  order-preserving with bits(0) == 0 and bits(d>0) >= 128 (values below
  1e-38 cannot occur: data gaps are > 1e-6).  So
  min over the field of (bitcast_i16(d) + n) == the argmax position n,
  computed entirely in int16.

Sharding: pure data parallel over tokens; each of the 8 cores gets
131072/8 = 16384 tokens as its own ExternalInput (plus two tiny
replicated constant tensors, DMA'd like inputs).

Engine choice (from per-op HW microbenchmarks): everything computes on
the Vector engine.  Offloading passes to GpSimd/ACT lowers DVE busy
time but concurrent GpSimd streaming makes co-scheduled DVE ops hit a
nondeterministic ~17x slow path and adds cross-engine bubbles; the
single-engine schedule overlaps DMA nearly perfectly and measured
fastest.  Small algebra runs as tensor_tensor against constant tiles
(tensor_scalar with strided inputs has the same slow-path hazard);
constants are shipped from the host as extra DMA inputs.  Chunks are
tapered [T/2, 3T/2 | 3T/2, T/2] to shorten pipeline fill/drain, and
each store is split in two for finer DMA interleave.
"""

import sys

if "/opt/trn_rl_repo" not in sys.path:
    sys.path.insert(0, "/opt/trn_rl_repo")

import numpy as np

B, S, D = 16, 8192, 128
N_CORES = 8
TOK = B * S                      # 131072 tokens
TOK_PER_CORE = TOK // N_CORES    # 16384
P = 128                          # SBUF partitions

OUT_LO, OUT_HI = 68, 84

T_CHUNK = 32
GROUP = 2
GT_ = GROUP * T_CHUNK

def const_layout(t_per_chunk=T_CHUNK, group=GROUP):
    """int16 constant block layout (per partition): n-pattern then 1/2/3/16.

    Chunks are tapered [T/2, 3T/2 | 3T/2, T/2]; the n-pattern must cover
    the largest chunk (3T/2).
    """
    GT = group * t_per_chunk
    t_max = t_per_chunk + t_per_chunk // 4
    idx_len = t_max * 64
    offs = {
        "idx": 0,
        "one": idx_len,
        "two": idx_len + GT,
        "three": idx_len + 2 * GT,
        "sixteen": idx_len + 3 * GT,
    }
    return offs, idx_len + 4 * GT


def make_const_inputs(t_per_chunk=T_CHUNK, group=GROUP):
    GT = group * t_per_chunk
    t_max = t_per_chunk + t_per_chunk // 4
    offs, ci_len = const_layout(t_per_chunk, group)
    ci = np.zeros((P, ci_len), np.int16)
    ci[:, 0 : t_max * 64] = np.tile(np.arange(16, dtype=np.int16), t_max * 4)
    for name, val in (("one", 1), ("two", 2), ("three", 3), ("sixteen", 16)):
        o = offs[name]
        ci[:, o : o + GT] = val
    cf = np.full((P, t_max * 4), 0.5, np.float32)
    return ci, cf


def build_program(tok_per_core=TOK_PER_CORE, t_per_chunk=T_CHUNK, group=GROUP):
    """Build + compile the single-core SPMD Bass program.

    The core's [tok_per_core, 128] slab is processed in chunks of 128*T
    tokens (contiguous DRAM block <-> SBUF tile [128, T*128]).  Heavy
    streaming passes run per chunk; small per-token algebra runs once per
    group of `group` chunks.
    """
    import concourse.bass as bass  # noqa: F401
    from concourse import bacc, mybir, tile

    f32 = mybir.dt.float32
    bf16 = mybir.dt.bfloat16
    i16 = mybir.dt.int16
    Op = mybir.AluOpType
    X = mybir.AxisListType.X

    T = t_per_chunk
    assert T % 4 == 0 and group == 2
    assert tok_per_core % (P * T * group) == 0
    n_groups = tok_per_core // (P * T * group)
    GT = group * T
    t_max = T + T // 4
    # tapered chunk sizes: smaller first chunk (fast pipeline fill) and
    # smaller last chunk (short drain); every group spans GT tokens/partition
    t_min = T - T // 4
    sched = [[t_min, t_max]] + [[t_max, t_min] for _ in range(n_groups - 1)]
    if n_groups > 2:
        for gsz in sched[1:-1]:
            gsz[0] = T
            gsz[1] = T
    offs, ci_len = const_layout(t_per_chunk, group)

    nc = bacc.Bacc(
        "TRN2",
        target_bir_lowering=False,
        debug=False,
        enable_asserts=True,
        num_devices=N_CORES,
    )
    x_dram = nc.dram_tensor("x", [tok_per_core, D], f32, kind="ExternalInput").ap()
    ci_dram = nc.dram_tensor("ci", [P, ci_len], i16, kind="ExternalInput").ap()
    cf_dram = nc.dram_tensor("cf", [P, t_max * 4], f32, kind="ExternalInput").ap()
    y_dram = nc.dram_tensor("y", [tok_per_core, D], f32, kind="ExternalOutput").ap()

    with tile.TileContext(nc) as tc:
        with (
            tc.tile_pool(name="consts", bufs=1) as cpool,
            tc.tile_pool(name="xtiles", bufs=5) as xpool,
            tc.tile_pool(name="big", bufs=3) as bp,
            tc.tile_pool(name="small", bufs=3) as sp,
        ):
            v = nc.vector
            g = nc.gpsimd

            # --- constants (DMA'd from host; first in the sync-ring FIFO,
            # they're tiny and everything downstream needs them) ------------
            cit = cpool.tile([P, ci_len], i16)
            nc.sync.dma_start(cit[:], ci_dram)
            cft = cpool.tile([P, t_max * 4], f32)
            nc.sync.dma_start(cft[:], cf_dram)

            idxi_full_max = cit[:, 0 : t_max * 64]

            def c1(off):  # [P, GT, 1] int16 const view
                return cit[:, off : off + GT].unsqueeze(2)

            ones, twos, threes, sixteens = (
                c1(offs["one"]), c1(offs["two"]),
                c1(offs["three"]), c1(offs["sixteen"]),
            )
            half4_max = cft.rearrange("p (t f) -> p t f", f=4)

            tok0 = 0
            for gi in range(n_groups):
                xts = []
                # group result tiles (interleaved [t, 4]), int16
                am_all = sp.tile([P, GT * 4], i16, name="am_all")
                am4 = am_all.rearrange("p (t g) -> p t g", g=4)
                fl_all = sp.tile([P, GT * 4], i16, name="fl_all")
                fl4 = fl_all.rearrange("p (t f) -> p t f", f=4)

                tws = []
                tw = 0
                for ci in range(group):
                    Tc = sched[gi][ci]
                    tws.append(tw)
                    chunk_tok = P * Tc
                    xt = xpool.tile([P, t_max * D], f32, name="xt")[:, 0 : Tc * D]
                    xts.append(xt)
                    src = x_dram[tok0 : tok0 + chunk_tok, :].rearrange(
                        "(p t) f -> p (t f)", p=P
                    )
                    nc.sync.dma_start(xt[:], src)

                    x3 = xt.rearrange("p (t f) -> p t f", f=D)
                    nib = x3[:, :, 4:68].rearrange("p t (g n) -> p t g n", n=16)

                    # field max (exact, f32)
                    rmax = bp.tile([P, t_max * 4], f32, name="rmax")[:, 0 : Tc * 4]
                    rmax3 = rmax.rearrange("p (t g) -> p t g", g=4)
                    v.tensor_reduce(rmax3, nib, axis=X, op=Op.max)

                    # d = max - x >= 0, as bf16 (fused (-1*x) + max on DVE)
                    dsub = bp.tile([P, t_max * 64], bf16, name="dsub")[:, 0 : Tc * 64]
                    dsub4 = dsub.rearrange("p (t g n) -> p t g n", g=4, n=16)
                    v.scalar_tensor_tensor(
                        dsub4,
                        nib,
                        -1.0,
                        rmax3.unsqueeze(3).broadcast_to([P, Tc, 4, 16]),
                        Op.mult,
                        Op.add,
                    )

                    # cand = bits(d) + n  (int16; == n exactly at max positions)
                    cand = bp.tile([P, t_max * 64], i16, name="cand")[:, 0 : Tc * 64]
                    v.tensor_tensor(
                        cand[:], dsub[:].bitcast(i16), idxi_full_max[:, 0 : Tc * 64],
                        Op.add,
                    )

                    # per-field argmax position via a 16->8->4->2->1 min tree:
                    # int16 tensor_tensor runs in 2x perf mode, unlike the
                    # always-1x tensor_reduce (halves the cycles of this pass)
                    c4 = cand.rearrange("p (t g n) -> p t g n", g=4, n=16)
                    m8 = bp.tile([P, t_max * 32], i16, name="m8")[
                        :, 0 : Tc * 32
                    ].rearrange("p (t g n) -> p t g n", g=4, n=8)
                    v.tensor_tensor(m8, c4[:, :, :, 0:8], c4[:, :, :, 8:16], Op.min)
                    m4 = bp.tile([P, t_max * 16], i16, name="m4")[
                        :, 0 : Tc * 16
                    ].rearrange("p (t g n) -> p t g n", g=4, n=4)
                    v.tensor_tensor(m4, m8[:, :, :, 0:4], m8[:, :, :, 4:8], Op.min)
                    m2 = bp.tile([P, t_max * 8], i16, name="m2")[
                        :, 0 : Tc * 8
                    ].rearrange("p (t g n) -> p t g n", g=4, n=2)
                    v.tensor_tensor(m2, m4[:, :, :, 0:2], m4[:, :, :, 2:4], Op.min)
                    v.tensor_tensor(
                        am4[:, tw : tw + Tc, :].unsqueeze(3),
                        m2[:, :, :, 0:1],
                        m2[:, :, :, 1:2],
                        Op.min,
                    )
                    # flags (>= 0.5) for cols 0..3 as int16 0/1
                    v.tensor_tensor(
                        fl4[:, tw : tw + Tc, :],
                        x3[:, :, 0:4],
                        half4_max[:, 0:Tc, :],
                        Op.is_ge,
                    )
                    tok0 += chunk_tok
                    tw += Tc

                # --- per-token algebra for the group, all int16.  Lo and hi
                # halves ride together as a [P, GT, 2] pair per op (am4 fields
                # are a_lo, a_hi, b_lo, b_hi) — halves the small-op count.
                mk = fl4[:, :, 0:1]
                ia = fl4[:, :, 1:2]
                io = fl4[:, :, 2:3]
                ix = fl4[:, :, 3:4]
                amL = am4[:, :, 0:2]
                amR = am4[:, :, 2:4]

                def t1(nm):
                    t_ = sp.tile([P, GT], i16, name=nm)
                    return t_.unsqueeze(2)   # [P, GT, 1]

                def t2(nm):
                    t_ = sp.tile([P, GT * 2], i16, name=nm)
                    return t_.rearrange("p (t h) -> p t h", h=2)  # [P, GT, 2]

                alpha = t1("alpha")          # 1 - is_and
                v.tensor_tensor(alpha, ones, ia, Op.subtract)
                s1 = t1("s1")                # 3 - is_or
                v.tensor_tensor(s1, threes, io, Op.subtract)
                s3 = t1("s3")                # is_or - 2
                v.tensor_tensor(s3, io, twos, Op.subtract)
                s2 = t1("s2")
                v.tensor_tensor(s2, ia, s1, Op.mult)
                beta = t1("beta")            # 1 / -1 / -2
                v.tensor_tensor(beta, s2, s3, Op.add)
                or1 = t1("or1")
                v.tensor_tensor(or1, ia, io, Op.bitwise_or)
                or2 = t1("or2")
                v.tensor_tensor(or2, or1, ix, Op.bitwise_or)
                acti = t1("acti")            # active = mark & any-flag
                v.tensor_tensor(acti, mk, or2, Op.bitwise_and)
                act16 = t1("act16")
                v.tensor_tensor(act16, acti, sixteens, Op.mult)
                goff = t1("goff")            # 16*(1-active)
                v.tensor_tensor(goff, sixteens, act16, Op.subtract)

                qi = t2("qi")                # a AND b, both halves
                v.tensor_tensor(qi, amL, amR, Op.bitwise_and)
                ss = t2("ss")                # a + b
                v.tensor_tensor(ss, amL, amR, Op.add)
                c1_ = t2("c1")
                v.tensor_tensor(c1_, ss, alpha.broadcast_to([P, GT, 2]), Op.mult)
                c2 = t2("c2")
                v.tensor_tensor(c2, qi, beta.broadcast_to([P, GT, 2]), Op.mult)
                res2 = t2("res2")            # op(a, b), 0..15
                v.tensor_tensor(res2, c1_, c2, Op.add)
                resg2 = t2("resg2")          # pushed out of 0..15 if inactive
                v.tensor_tensor(resg2, res2, goff.broadcast_to([P, GT, 2]), Op.add)

                # one-hot over the contiguous 32-feature output block 68:100:
                # lane (h, n) = 1 iff resg2[h] == n  (the 2x rides the accum)
                eq2 = sp.tile([P, GT * 32], bf16, name="eq2")
                eq4 = eq2.rearrange("p (t h n) -> p t h n", h=2, n=16)
                v.tensor_tensor(
                    eq4,
                    idxi_full_max[:, 0 : GT * 32].rearrange(
                        "p (t h n) -> p t h n", h=2, n=16
                    ),
                    resg2.unsqueeze(3).broadcast_to([P, GT, 2, 16]),
                    Op.is_equal,
                )
                eq3 = eq2.rearrange("p (t k) -> p t k", k=32)

                # --- accumulate into x and store, per chunk ----------------
                stok0 = tok0 - P * GT
                for ci in range(group):
                    Tc = sched[gi][ci]
                    tw = tws[ci]
                    x3 = xts[ci].rearrange("p (t f) -> p t f", f=D)
                    xs = x3[:, :, OUT_LO : OUT_LO + 32]
                    v.scalar_tensor_tensor(
                        xs,
                        eq3[:, tw : tw + Tc, :],
                        2.0,
                        xs,
                        Op.mult,
                        Op.add,
                    )
                    # two half-stores: finer DMA interleave, shorter tail
                    half = Tc // 2
                    dst3 = y_dram[stok0 : stok0 + P * Tc, :].rearrange(
                        "(p t) f -> p t f", p=P
                    )
                    src3 = x3
                    # stores ride the scalar HWDGE ring (qActDynamicHW) so they
                    # don't serialize behind loads on qSyncDynamicHW — each HW
                    # ring tops out ~200 GB/s, together they reach the HBM limit
                    for s in range(2):
                        nc.scalar.dma_start(
                            dst3[:, s * half : (s + 1) * half, :],
                            src3[:, s * half : (s + 1) * half, :],
                        )
                    stok0 += P * Tc

    nc.compile()
    return nc


_compiled = None


def _get_compiled():
    global _compiled
    if _compiled is None:
        _compiled = build_program()
    return _compiled


def run_on_hw(nc, shards, trace=False, t_per_chunk=T_CHUNK, group=GROUP, **kw):
    from concourse.bass_utils import run_bass_kernel_spmd

    ci, cf = make_const_inputs(t_per_chunk, group)
    return run_bass_kernel_spmd(
        nc,
        [{"x": s, "ci": ci, "cf": cf} for s in shards],
        list(range(N_CORES)),
        trace=trace,
        **kw,
    )


def kernel(x_bd, and_table=None, or_table=None, xor_table=None):
    x = np.ascontiguousarray(np.asarray(x_bd, dtype=np.float32)).reshape(TOK, D)
    shards = [
        np.ascontiguousarray(x[c * TOK_PER_CORE : (c + 1) * TOK_PER_CORE])
        for c in range(N_CORES)
    ]
    nc = _get_compiled()
    res = run_on_hw(nc, shards)
    out = np.concatenate([res.results[c]["y"] for c in range(N_CORES)], axis=0)
    return out.reshape(B, S, D).astype(np.float32)



# revision 11
# speedup vs baseline: 1.3173x; 1.0286x over previous
"""Trainium2 Bass kernel for nn_ByteBitwiseFFN.

Reference semantics (per token, D=128 features):
  a = argmax(x[4:20]) + 16*argmax(x[20:36])
  b = argmax(x[36:52]) + 16*argmax(x[52:68])
  res = AND/OR/XOR LUT[a,b] picked by flags x[1]>0.5 / x[2]>0.5 / x[3]>0.5
        (priority AND, OR, XOR; XOR value also used when no flag set)
  active = (x[0]>=0.5) & any-flag; w = active ? 2 : 0
  out = x; out[68 + (res&15)] += w; out[84 + (res>>4)] += w

Key identities:
* Bitwise ops factor over nibbles, so the 256x256 LUTs are never needed:
  res&15 = op(a_lo, b_lo), res>>4 = op(a_hi, b_hi), and for 4-bit operands
  op(u, v) = alpha*(u+v) + beta*(u AND v) with (alpha, beta) =
  (0,1) AND / (1,-1) OR / (1,-2) XOR.  The AND is one int16 bitwise_and.
* Compare-free first-occurrence argmax via the bf16 bit pattern:
  d = max - x >= 0, and for non-negative bf16 the raw bit pattern is
  order-preserving with bits(0) == 0 and bits(d>0) >= 128 (values below
  1e-38 cannot occur: data gaps are > 1e-6).  So
  min over the field of (bitcast_i16(d) + n) == the argmax position n,
  computed entirely in int16.  The min runs as a 16->8->4->2->1
  tensor_tensor tree (2x perf mode) instead of the always-1x
  tensor_reduce.

I/O: only features 0:100 of each token are ever read (flags 0:4, nibble
fields 4:68, accumulate base 68:100) and only 68:100 is ever written;
features 0:68 and 100:128 pass through unchanged.  The host pre-slices
the input to a packed [tok, 100] slab per core and merges the packed
[tok, 32] device output back into a host-side copy of x.  Device HBM
traffic per core drops from 16.8 MB to 10.5 MB, and every DMA stays
fully contiguous per partition (no strided-descriptor penalty).

Sharding: pure data parallel over tokens; each of the 8 cores gets
131072/8 = 16384 tokens as its own ExternalInput (plus two tiny
replicated constant tensors, DMA'd like inputs).

Scheduling: chunk loads alternate between the two HWDGE rings
(qSyncDynamicHW / qActDynamicHW) so two load streams run concurrently;
packed stores ride the ring opposite their chunk's load.  Each group's
post-argmax block (algebra, one-hot, accumulate, store) is wrapped in
tc.high_priority() so the Tile scheduler interleaves it with the next
chunks' heavy passes instead of deferring all stores to the end.
Chunks are tapered [3T/4, 5T/4 | 5T/4, 3T/4] for pipeline fill/drain.
"""

import sys

if "/opt/trn_rl_repo" not in sys.path:
    sys.path.insert(0, "/opt/trn_rl_repo")

import numpy as np

B, S, D = 16, 8192, 128
N_CORES = 8
TOK = B * S                      # 131072 tokens
TOK_PER_CORE = TOK // N_CORES    # 16384
P = 128                          # SBUF partitions

OUT_LO, OUT_HI = 68, 84
D_IN = 100                       # features 0:100 are read
D_OUT = 32                       # features 68:100 are written

T_CHUNK = 32
GROUP = 2
GT_ = GROUP * T_CHUNK

def const_layout(t_per_chunk=T_CHUNK, group=GROUP):
    """int16 constant block layout (per partition): n-pattern then 1/2/3/16.

    Chunks are tapered [3T/4, 5T/4 | 5T/4, 3T/4]; the n-pattern must cover
    the largest chunk (5T/4).
    """
    GT = group * t_per_chunk
    t_max = t_per_chunk + t_per_chunk // 4
    idx_len = t_max * 64
    offs = {
        "idx": 0,
        "one": idx_len,
        "two": idx_len + GT,
        "three": idx_len + 2 * GT,
        "sixteen": idx_len + 3 * GT,
    }
    return offs, idx_len + 4 * GT


def make_const_inputs(t_per_chunk=T_CHUNK, group=GROUP):
    GT = group * t_per_chunk
    t_max = t_per_chunk + t_per_chunk // 4
    offs, ci_len = const_layout(t_per_chunk, group)
    ci = np.zeros((P, ci_len), np.int16)
    ci[:, 0 : t_max * 64] = np.tile(np.arange(16, dtype=np.int16), t_max * 4)
    for name, val in (("one", 1), ("two", 2), ("three", 3), ("sixteen", 16)):
        o = offs[name]
        ci[:, o : o + GT] = val
    cf = np.full((P, t_max * 4), 0.5, np.float32)
    return ci, cf


def build_program(tok_per_core=TOK_PER_CORE, t_per_chunk=T_CHUNK, group=GROUP):
    """Build + compile the single-core SPMD Bass program.

    The core's packed [tok_per_core, 100] slab is processed in chunks of
    128*T tokens (contiguous DRAM block <-> SBUF tile [128, T*100]).
    Heavy streaming passes run per chunk; small per-token algebra runs
    once per group of `group` chunks.
    """
    import concourse.bass as bass  # noqa: F401
    from concourse import bacc, mybir, tile

    f32 = mybir.dt.float32
    bf16 = mybir.dt.bfloat16
    i16 = mybir.dt.int16
    Op = mybir.AluOpType
    X = mybir.AxisListType.X

    T = t_per_chunk
    assert T % 4 == 0 and group == 2
    assert tok_per_core % (P * T * group) == 0
    n_groups = tok_per_core // (P * T * group)
    GT = group * T
    t_max = T + T // 4
    t_min = T - T // 4
    # tapered chunk sizes: smaller first chunk (fast pipeline fill) and
    # smaller last chunk (short drain); every group spans GT tokens/partition
    sched = [[t_min, t_max]] + [[t_max, t_min] for _ in range(n_groups - 1)]
    offs, ci_len = const_layout(t_per_chunk, group)

    nc = bacc.Bacc(
        "TRN2",
        target_bir_lowering=False,
        debug=False,
        enable_asserts=True,
        num_devices=N_CORES,
    )
    x_dram = nc.dram_tensor("x", [tok_per_core, D_IN], f32, kind="ExternalInput").ap()
    ci_dram = nc.dram_tensor("ci", [P, ci_len], i16, kind="ExternalInput").ap()
    cf_dram = nc.dram_tensor("cf", [P, t_max * 4], f32, kind="ExternalInput").ap()
    y_dram = nc.dram_tensor(
        "y", [tok_per_core, D_OUT], f32, kind="ExternalOutput"
    ).ap()

    with tile.TileContext(nc) as tc:
        with (
            tc.tile_pool(name="consts", bufs=1) as cpool,
            tc.tile_pool(name="xtiles", bufs=5) as xpool,
            tc.tile_pool(name="big", bufs=3) as bp,
            tc.tile_pool(name="small", bufs=3) as sp,
        ):
            v = nc.vector

            # --- constants (DMA'd from host; first in the sync-ring FIFO,
            # they're tiny and everything downstream needs them) ------------
            cit = cpool.tile([P, ci_len], i16)
            nc.sync.dma_start(cit[:], ci_dram)
            cft = cpool.tile([P, t_max * 4], f32)
            nc.sync.dma_start(cft[:], cf_dram)

            idxi_full_max = cit[:, 0 : t_max * 64]

            def c1(off):  # [P, GT, 1] int16 const view
                return cit[:, off : off + GT].unsqueeze(2)

            ones, twos, threes, sixteens = (
                c1(offs["one"]), c1(offs["two"]),
                c1(offs["three"]), c1(offs["sixteen"]),
            )
            half4_max = cft.rearrange("p (t f) -> p t f", f=4)

            # chunk loads alternate between the two HWDGE rings; the packed
            # store for a chunk rides the opposite ring
            rings = [nc.sync, nc.scalar]

            tok0 = 0
            gchunk = 0
            for gi in range(n_groups):
                xts = []
                # group result tiles (interleaved [t, 4]), int16
                am_all = sp.tile([P, GT * 4], i16, name="am_all")
                am4 = am_all.rearrange("p (t g) -> p t g", g=4)
                fl_all = sp.tile([P, GT * 4], i16, name="fl_all")
                fl4 = fl_all.rearrange("p (t f) -> p t f", f=4)

                tws = []
                store_rings = []
                tw = 0
                for ci in range(group):
                    Tc = sched[gi][ci]
                    tws.append(tw)
                    store_rings.append(rings[(gchunk + 1) % 2])
                    chunk_tok = P * Tc
                    xt = xpool.tile([P, t_max * D_IN], f32, name="xt")[
                        :, 0 : Tc * D_IN
                    ]
                    xts.append(xt)
                    src = x_dram[tok0 : tok0 + chunk_tok, :].rearrange(
                        "(p t) f -> p (t f)", p=P
                    )
                    rings[gchunk % 2].dma_start(xt[:], src)
                    gchunk += 1

                    x3 = xt.rearrange("p (t f) -> p t f", f=D_IN)
                    nib = x3[:, :, 4:68].rearrange("p t (g n) -> p t g n", n=16)

                    # field max (exact, f32)
                    rmax = bp.tile([P, t_max * 4], f32, name="rmax")[:, 0 : Tc * 4]
                    rmax3 = rmax.rearrange("p (t g) -> p t g", g=4)
                    v.tensor_reduce(rmax3, nib, axis=X, op=Op.max)

                    # d = max - x >= 0, as bf16 (fused (-1*x) + max on DVE)
                    dsub = bp.tile([P, t_max * 64], bf16, name="dsub")[:, 0 : Tc * 64]
                    dsub4 = dsub.rearrange("p (t g n) -> p t g n", g=4, n=16)
                    v.scalar_tensor_tensor(
                        dsub4,
                        nib,
                        -1.0,
                        rmax3.unsqueeze(3).broadcast_to([P, Tc, 4, 16]),
                        Op.mult,
                        Op.add,
                    )

                    # cand = bits(d) + n  (int16; == n exactly at max positions)
                    cand = bp.tile([P, t_max * 64], i16, name="cand")[:, 0 : Tc * 64]
                    v.tensor_tensor(
                        cand[:], dsub[:].bitcast(i16), idxi_full_max[:, 0 : Tc * 64],
                        Op.add,
                    )

                    # per-field argmax position via a 16->8->4->2->1 min tree:
                    # int16 tensor_tensor runs in 2x perf mode, unlike the
                    # always-1x tensor_reduce (halves the cycles of this pass)
                    c4 = cand.rearrange("p (t g n) -> p t g n", g=4, n=16)
                    m8 = bp.tile([P, t_max * 32], i16, name="m8")[
                        :, 0 : Tc * 32
                    ].rearrange("p (t g n) -> p t g n", g=4, n=8)
                    v.tensor_tensor(m8, c4[:, :, :, 0:8], c4[:, :, :, 8:16], Op.min)
                    m4 = bp.tile([P, t_max * 16], i16, name="m4")[
                        :, 0 : Tc * 16
                    ].rearrange("p (t g n) -> p t g n", g=4, n=4)
                    v.tensor_tensor(m4, m8[:, :, :, 0:4], m8[:, :, :, 4:8], Op.min)
                    m2 = bp.tile([P, t_max * 8], i16, name="m2")[
                        :, 0 : Tc * 8
                    ].rearrange("p (t g n) -> p t g n", g=4, n=2)
                    v.tensor_tensor(m2, m4[:, :, :, 0:2], m4[:, :, :, 2:4], Op.min)
                    v.tensor_tensor(
                        am4[:, tw : tw + Tc, :].unsqueeze(3),
                        m2[:, :, :, 0:1],
                        m2[:, :, :, 1:2],
                        Op.min,
                    )
                    # flags (>= 0.5) for cols 0..3 as int16 0/1
                    v.tensor_tensor(
                        fl4[:, tw : tw + Tc, :],
                        x3[:, :, 0:4],
                        half4_max[:, 0:Tc, :],
                        Op.is_ge,
                    )
                    tok0 += chunk_tok
                    tw += Tc

                # --- per-token algebra for the group, all int16.  Lo and hi
                # halves ride together as a [P, GT, 2] pair per op (am4
                # fields are a_lo, a_hi, b_lo, b_hi).  High priority: the
                # scheduler must prefer this (and the stores it feeds) over
                # the next group's heavy passes once deps are ready.
                with tc.high_priority():
                    mk = fl4[:, :, 0:1]
                    ia = fl4[:, :, 1:2]
                    io = fl4[:, :, 2:3]
                    ix = fl4[:, :, 3:4]
                    amL = am4[:, :, 0:2]
                    amR = am4[:, :, 2:4]

                    def t1(nm):
                        t_ = sp.tile([P, GT], i16, name=nm)
                        return t_.unsqueeze(2)   # [P, GT, 1]

                    def t2(nm):
                        t_ = sp.tile([P, GT * 2], i16, name=nm)
                        return t_.rearrange("p (t h) -> p t h", h=2)

                    alpha = t1("alpha")          # 1 - is_and
                    v.tensor_tensor(alpha, ones, ia, Op.subtract)
                    s1 = t1("s1")                # 3 - is_or
                    v.tensor_tensor(s1, threes, io, Op.subtract)
                    s3 = t1("s3")                # is_or - 2
                    v.tensor_tensor(s3, io, twos, Op.subtract)
                    s2 = t1("s2")
                    v.tensor_tensor(s2, ia, s1, Op.mult)
                    beta = t1("beta")            # 1 / -1 / -2
                    v.tensor_tensor(beta, s2, s3, Op.add)
                    or1 = t1("or1")
                    v.tensor_tensor(or1, ia, io, Op.bitwise_or)
                    or2 = t1("or2")
                    v.tensor_tensor(or2, or1, ix, Op.bitwise_or)
                    acti = t1("acti")            # active = mark & any-flag
                    v.tensor_tensor(acti, mk, or2, Op.bitwise_and)
                    act16 = t1("act16")
                    v.tensor_tensor(act16, acti, sixteens, Op.mult)
                    goff = t1("goff")            # 16*(1-active)
                    v.tensor_tensor(goff, sixteens, act16, Op.subtract)

                    qi = t2("qi")                # a AND b, both halves
                    v.tensor_tensor(qi, amL, amR, Op.bitwise_and)
                    ss = t2("ss")                # a + b
                    v.tensor_tensor(ss, amL, amR, Op.add)
                    c1_ = t2("c1")
                    v.tensor_tensor(
                        c1_, ss, alpha.broadcast_to([P, GT, 2]), Op.mult
                    )
                    c2 = t2("c2")
                    v.tensor_tensor(
                        c2, qi, beta.broadcast_to([P, GT, 2]), Op.mult
                    )
                    res2 = t2("res2")            # op(a, b), 0..15
                    v.tensor_tensor(res2, c1_, c2, Op.add)
                    resg2 = t2("resg2")          # pushed out of 0..15 if inactive
                    v.tensor_tensor(
                        resg2, res2, goff.broadcast_to([P, GT, 2]), Op.add
                    )

                    # one-hot over the 32-feature output block 68:100:
                    # lane (h, n) = 1 iff resg2[h] == n (the 2x rides accum)
                    eq2 = sp.tile([P, GT * 32], bf16, name="eq2")
                    eq4 = eq2.rearrange("p (t h n) -> p t h n", h=2, n=16)
                    v.tensor_tensor(
                        eq4,
                        idxi_full_max[:, 0 : GT * 32].rearrange(
                            "p (t h n) -> p t h n", h=2, n=16
                        ),
                        resg2.unsqueeze(3).broadcast_to([P, GT, 2, 16]),
                        Op.is_equal,
                    )
                    eq3 = eq2.rearrange("p (t k) -> p t k", k=32)

                    # --- accumulate into the packed output tile and store --
                    stok0 = tok0 - P * GT
                    for ci in range(group):
                        Tc = sched[gi][ci]
                        tw = tws[ci]
                        x3 = xts[ci].rearrange("p (t f) -> p t f", f=D_IN)
                        yt = xpool.tile([P, t_max * D_OUT], f32, name="yt")[
                            :, 0 : Tc * D_OUT
                        ]
                        yt3 = yt.rearrange("p (t f) -> p t f", f=D_OUT)
                        v.scalar_tensor_tensor(
                            yt3,
                            eq3[:, tw : tw + Tc, :],
                            2.0,
                            x3[:, :, OUT_LO : OUT_LO + D_OUT],
                            Op.mult,
                            Op.add,
                        )
                        dst = y_dram[stok0 : stok0 + P * Tc, :].rearrange(
                            "(p t) f -> p (t f)", p=P
                        )
                        store_rings[ci].dma_start(dst, yt[:])
                        stok0 += P * Tc

    nc.compile()
    return nc


_compiled = None


def _get_compiled():
    global _compiled
    if _compiled is None:
        _compiled = build_program()
    return _compiled


def run_on_hw(nc, shards, trace=False, t_per_chunk=T_CHUNK, group=GROUP, **kw):
    """shards: per-core [TOK_PER_CORE, 128] f32 rows (full feature dim);
    the packed [tok, 100] device input is sliced out here."""
    from concourse.bass_utils import run_bass_kernel_spmd

    ci, cf = make_const_inputs(t_per_chunk, group)
    feeds = [
        {"x": np.ascontiguousarray(s[:, 0:D_IN]), "ci": ci, "cf": cf}
        for s in shards
    ]
    return run_bass_kernel_spmd(
        nc,
        feeds,
        list(range(N_CORES)),
        trace=trace,
        **kw,
    )


def kernel(x_bd, and_table=None, or_table=None, xor_table=None):
    x = np.ascontiguousarray(np.asarray(x_bd, dtype=np.float32)).reshape(TOK, D)
    shards = [
        x[c * TOK_PER_CORE : (c + 1) * TOK_PER_CORE] for c in range(N_CORES)
    ]
    nc = _get_compiled()
    res = run_on_hw(nc, shards)
    out = x.copy()
    y = np.concatenate([res.results[c]["y"] for c in range(N_CORES)], axis=0)
    out[:, OUT_LO : OUT_LO + D_OUT] = y
    return out.reshape(B, S, D).astype(np.float32)
